# revision 2
# baseline (speedup 1.0000x reference)
"""Exponential smoothing EMA on 8 TRN2 cores, v3: transposed fp16 layout,
chunked scans for overlap.

Same math as v2: host supplies x^T (B, D, T) fp16; per core 8 chains
(2 batches x 4 d-chunks of 128 channels); each chain scans
s'_t = w*s'_{t-1} + x_t in TC-sized chunks on the vector engine (state fp32
internally, chained via the previous chunk's last column), with initial
state x_0/a so that s = a*s' and s_0 = x_0 exactly. Scalar engine does the
a-postscale per chunk; output DMA'd per chunk in transposed fp16.
"""

from contextlib import ExitStack

import numpy as np

import concourse.bass as bass
import concourse.tile as tile
from concourse import bacc, mybir
from concourse.bass_utils import run_bass_kernel_spmd

B, T, D = 16, 4096, 512
NCORES = 8
BL = B // NCORES
P = 128
ND = D // P
CHUNKS = (1024, 2048, 1024)   # per-chain scan chunk sizes (sum = T)
TCMAX = max(CHUNKS)

FP32 = mybir.dt.float32
FP16 = mybir.dt.float16


def build_program(bl: int = BL, t: int = T) -> bacc.Bacc:
    nc = bacc.Bacc(
        "TRN2",
        target_bir_lowering=False,
        debug=False,
        enable_asserts=False,
        num_devices=NCORES,
    )
    x = nc.dram_tensor("x", (bl, D, t), FP16, kind="ExternalInput").ap()
    alpha = nc.dram_tensor("alpha", (1, 1, D), FP32, kind="ExternalInput").ap()
    y = nc.dram_tensor("y", (bl, D, t), FP16, kind="ExternalOutput").ap()

    with tile.TileContext(nc) as tc, ExitStack() as ctx:
        const_pool = ctx.enter_context(tc.tile_pool(name="const", bufs=1))
        xn_pool = ctx.enter_context(tc.tile_pool(name="xn", bufs=8))
        s_pool = ctx.enter_context(tc.tile_pool(name="s", bufs=6))
        y_pool = ctx.enter_context(tc.tile_pool(name="y", bufs=4))

        alpha_sb = const_pool.tile([P, ND], FP32)
        nc.sync.dma_start(alpha_sb[:], alpha.rearrange("o u (j p) -> (o u p) j", p=P))
        a_sb = const_pool.tile([P, ND], FP32)
        nc.scalar.activation(a_sb[:], alpha_sb[:], mybir.ActivationFunctionType.Sigmoid)
        w_sb = const_pool.tile([P, ND], FP32)
        nc.scalar.activation(
            w_sb[:], alpha_sb[:], mybir.ActivationFunctionType.Sigmoid, scale=-1.0
        )
        inva_sb = const_pool.tile([P, ND], FP32)
        nc.vector.reciprocal(inva_sb[:], a_sb[:])
        zeros1 = const_pool.tile([P, 1], FP16)
        nc.vector.memset(zeros1[:], 0.0)

        # ones donor on the (otherwise idle) gpsimd engine, then per-chunk w
        # broadcasts on the scalar engine; wb0's first half unblocks chunk 0.
        ones16 = const_pool.tile([P, TCMAX], FP16)
        nc.gpsimd.memset(ones16[:], 1.0)
        wbs = []
        for j in range(ND):
            # Built in halves so chunk 0's scan only waits on the first half.
            wb = const_pool.tile([P, TCMAX], FP16, tag=f"wb{j}")
            h = TCMAX // 2
            nc.scalar.mul(wb[:, 0:h], ones16[:, 0:h], w_sb[:, j : j + 1])
            nc.scalar.mul(wb[:, h:TCMAX], ones16[:, h:TCMAX], w_sb[:, j : j + 1])
            wbs.append(wb)

        for j in range(ND):
            for b in range(bl):
                s_prev = None
                t0 = 0
                for k, tc in enumerate(CHUNKS):
                    xn = xn_pool.tile([P, tc], FP16, tag=f"xn{k}")
                    nc.sync.dma_start(
                        xn[:], x[b, j * P : (j + 1) * P, t0 : t0 + tc]
                    )
                    if k == 0:
                        # init = x_0 / a on the vector engine (tiny op, keeps
                        # the chain start off the busy scalar queue).
                        init = const_pool.tile([P, 1], FP16, tag=f"init{j}_{b}")
                        nc.vector.scalar_tensor_tensor(
                            init[:],
                            xn[:, 0:1],
                            inva_sb[:, j : j + 1],
                            zeros1[:],
                            mybir.AluOpType.mult,
                            mybir.AluOpType.add,
                        )
                    else:
                        init = s_prev[:, -1:]
                    s = s_pool.tile([P, tc], FP16, tag=f"s{k}")
                    nc.vector.tensor_tensor_scan(
                        s[:],
                        wbs[j][:, 0:tc],
                        xn[:],
                        init,
                        mybir.AluOpType.mult,
                        mybir.AluOpType.add,
                    )
                    s_prev = s

                    yo = y_pool.tile([P, tc], FP16, tag=f"y{k}")
                    last_unit = j == ND - 1 and b == bl - 1
                    if last_unit and k == len(CHUNKS) - 1:
                        # Final chunk: scale on DVE right after its own scan —
                        # no cross-engine hop on the critical tail.
                        nc.vector.tensor_scalar_mul(
                            yo[:], s[:], a_sb[:, j : j + 1]
                        )
                    else:
                        nc.scalar.mul(yo[:], s[:], a_sb[:, j : j + 1])
                    if j == ND - 1:
                        nc.sync.dma_start(
                            y[b, j * P : (j + 1) * P, t0 : t0 + tc], yo[:]
                        )
                    else:
                        nc.gpsimd.dma_start(
                            y[b, j * P : (j + 1) * P, t0 : t0 + tc], yo[:]
                        )
                    t0 += tc

    nc.compile()
    return nc


_prog = None


def kernel(x, alpha):
    global _prog
    if _prog is None:
        _prog = build_program()
    x = np.asarray(x, dtype=np.float32)
    alpha = np.ascontiguousarray(np.asarray(alpha, dtype=np.float32))
    assert x.shape == (B, T, D) and alpha.shape == (1, 1, D)
    xt = np.ascontiguousarray(x.transpose(0, 2, 1).astype(np.float16))
    in_maps = [
        {"x": xt[i * BL : (i + 1) * BL], "alpha": alpha}
        for i in range(NCORES)
    ]
    res = run_bass_kernel_spmd(_prog, in_maps, core_ids=list(range(NCORES)))
    yt = np.concatenate([r["y"] for r in res.results], axis=0)
    return np.ascontiguousarray(yt.transpose(0, 2, 1).astype(np.float32))


# revision 3
# speedup vs baseline: 1.0246x; 1.0246x over previous
"""Exponential smoothing EMA on 8 TRN2 cores, v3: transposed fp16 layout,
chunked scans for overlap.

Same math as v2: host supplies x^T (B, D, T) fp16; per core 8 chains
(2 batches x 4 d-chunks of 128 channels); each chain scans
s'_t = w*s'_{t-1} + x_t in TC-sized chunks on the vector engine (state fp32
internally, chained via the previous chunk's last column), with initial
state x_0/a so that s = a*s' and s_0 = x_0 exactly. Scalar engine does the
a-postscale per chunk; output DMA'd per chunk in transposed fp16.
"""

from contextlib import ExitStack

import numpy as np

import concourse.bass as bass
import concourse.tile as tile
from concourse import bacc, mybir
from concourse.bass_utils import run_bass_kernel_spmd

B, T, D = 16, 4096, 512
NCORES = 8
BL = B // NCORES
P = 128
ND = D // P
# chains 0-6 use two chunks (fewer per-instruction overheads); the last
# chain keeps a small final chunk so the tail postscale+store is short.
CHUNKS_MID = (1024, 3072)
CHUNKS_LAST = (1024, 2048, 1024)
TCMAX = 3072

FP32 = mybir.dt.float32
FP16 = mybir.dt.float16


def build_program(bl: int = BL, t: int = T) -> bacc.Bacc:
    nc = bacc.Bacc(
        "TRN2",
        target_bir_lowering=False,
        debug=False,
        enable_asserts=False,
        num_devices=NCORES,
    )
    x = nc.dram_tensor("x", (bl, D, t), FP16, kind="ExternalInput").ap()
    alpha = nc.dram_tensor("alpha", (1, 1, D), FP32, kind="ExternalInput").ap()
    y = nc.dram_tensor("y", (bl, D, t), FP16, kind="ExternalOutput").ap()

    with tile.TileContext(nc) as tc, ExitStack() as ctx:
        const_pool = ctx.enter_context(tc.tile_pool(name="const", bufs=1))
        xn_pool = ctx.enter_context(tc.tile_pool(name="xn", bufs=3))
        s_pool = ctx.enter_context(tc.tile_pool(name="s", bufs=3))
        y_pool = ctx.enter_context(tc.tile_pool(name="y", bufs=3))

        alpha_sb = const_pool.tile([P, ND], FP32)
        nc.sync.dma_start(alpha_sb[:], alpha.rearrange("o u (j p) -> (o u p) j", p=P))
        a_sb = const_pool.tile([P, ND], FP32)
        nc.scalar.activation(a_sb[:], alpha_sb[:], mybir.ActivationFunctionType.Sigmoid)
        w_sb = const_pool.tile([P, ND], FP32)
        nc.scalar.activation(
            w_sb[:], alpha_sb[:], mybir.ActivationFunctionType.Sigmoid, scale=-1.0
        )
        inva_sb = const_pool.tile([P, ND], FP32)
        nc.vector.reciprocal(inva_sb[:], a_sb[:])
        zeros1 = const_pool.tile([P, 1], FP16)
        nc.vector.memset(zeros1[:], 0.0)

        # ones donor on the (otherwise idle) gpsimd engine, then per-chunk w
        # broadcasts on the scalar engine; wb0's first half unblocks chunk 0.
        ones16 = const_pool.tile([P, TCMAX], FP16)
        nc.gpsimd.memset(ones16[:], 1.0)
        wbs = []
        for j in range(ND):
            # Built in halves so chunk 0's scan only waits on the first half.
            # Chain 0's first half rides the (idle until scan 0) vector engine
            # so scan 0 isn't queued behind the scalar sigmoid chain.
            wb = const_pool.tile([P, TCMAX], FP16, tag=f"wb{j}")
            h = 1024  # chunk 0 only needs the first 1024 columns
            nc.scalar.mul(wb[:, 0:h], ones16[:, 0:h], w_sb[:, j : j + 1])
            nc.scalar.mul(wb[:, h:TCMAX], ones16[:, h:TCMAX], w_sb[:, j : j + 1])
            wbs.append(wb)

        for j in range(ND):
            for b in range(bl):
                last_chain = j == ND - 1 and b == bl - 1
                chunks = CHUNKS_LAST if last_chain else CHUNKS_MID
                s_prev = None
                t0 = 0
                for k, tc in enumerate(chunks):
                    xn = xn_pool.tile([P, tc], FP16, tag=f"xn{tc}")
                    nc.sync.dma_start(
                        xn[:], x[b, j * P : (j + 1) * P, t0 : t0 + tc]
                    )
                    if k == 0:
                        # init = x_0 / a on the vector engine (tiny op, keeps
                        # the chain start off the busy scalar queue).
                        init = const_pool.tile([P, 1], FP16, tag=f"init{j}_{b}")
                        nc.vector.scalar_tensor_tensor(
                            init[:],
                            xn[:, 0:1],
                            inva_sb[:, j : j + 1],
                            zeros1[:],
                            mybir.AluOpType.mult,
                            mybir.AluOpType.add,
                        )
                    else:
                        init = s_prev[:, -1:]
                    s = s_pool.tile([P, tc], FP16, tag=f"s{tc}")
                    nc.vector.tensor_tensor_scan(
                        s[:],
                        wbs[j][:, 0:tc],
                        xn[:],
                        init,
                        mybir.AluOpType.mult,
                        mybir.AluOpType.add,
                    )
                    s_prev = s

                    yo = y_pool.tile([P, tc], FP16, tag=f"y{tc}")
                    if last_chain and k == len(chunks) - 1:
                        # Final chunk: scale on DVE right after its own scan —
                        # no cross-engine hop on the critical tail.
                        nc.vector.tensor_scalar_mul(
                            yo[:], s[:], a_sb[:, j : j + 1]
                        )
                    else:
                        nc.scalar.mul(yo[:], s[:], a_sb[:, j : j + 1])
                    if j >= ND - 2:
                        nc.sync.dma_start(
                            y[b, j * P : (j + 1) * P, t0 : t0 + tc], yo[:]
                        )
                    else:
                        nc.gpsimd.dma_start(
                            y[b, j * P : (j + 1) * P, t0 : t0 + tc], yo[:]
                        )
                    t0 += tc

    nc.compile()
    return nc


_prog = None


def kernel(x, alpha):
    global _prog
    if _prog is None:
        _prog = build_program()
    x = np.asarray(x, dtype=np.float32)
    alpha = np.ascontiguousarray(np.asarray(alpha, dtype=np.float32))
    assert x.shape == (B, T, D) and alpha.shape == (1, 1, D)
    xt = np.ascontiguousarray(x.transpose(0, 2, 1).astype(np.float16))
    in_maps = [
        {"x": xt[i * BL : (i + 1) * BL], "alpha": alpha}
        for i in range(NCORES)
    ]
    res = run_bass_kernel_spmd(_prog, in_maps, core_ids=list(range(NCORES)))
    yt = np.concatenate([r["y"] for r in res.results], axis=0)
    return np.ascontiguousarray(yt.transpose(0, 2, 1).astype(np.float32))
